# revision 27
# baseline (speedup 1.0000x reference)
"""Trainium2 Bass kernel for GNN link-prediction BCE loss.

loss = mean over 3M edges of BCE-with-logits(dot(h[src], h[dst]), label)
     = [ sum_pos softplus(-s) + sum_neg softplus(+s) ] / 3M

The workload is descriptor-rate bound on the SWDGE gather path (measured
~1.8ns fixed + ~0.9ns/256B per descriptor, 1024 descs/call max), so the
kernel minimizes DESCRIPTOR COUNT, not call count or bytes.

Strategy (8 NeuronCores, SPMD):
 - h is cast to fp16 and repacked into 16 segments of 31250 rows (+1s
   special pad row per segment); pad edges hit (+1,+1) rows -> score 128
   -> exactly 0 loss with sigma=-1.
 - Diagonal bucket assignment: bucket (s_seg, d_seg) goes wholly to core
   ((d_seg-s_seg)%16)//2, giving every core 32 whole buckets. All cores
   run the IDENTICAL program: src gathers address the plain table, dst
   gathers a per-core table whose segments are rotated by 2c, making the
   schedule core-independent while the data differs.
 - Run-length-compressed gathers: within each bucket, dst values are
   sorted and packed into descriptors covering 4/2/1 CONSECUTIVE rows
   (elem_size=L*256B with elem_step=256B overlapping-window APs), so one
   descriptor serves up to 4 edges. Shared per-position class quotas
   (min across cores, excess demoted 4->2->1) keep the program SPMD.
 - The leftover dst-singles region is re-sorted by src value and the SRC
   side is run-length compressed there the same way (the dst side of
   those slots stays per-edge singles).
 - Macro-tiles of 4096 edge slots: gathers fill src/dst row tiles, DVE
   multiplies + reduces to fp16 scores, scalar engine computes stable
   softplus ln(1+exp(-|s|)) + relu(sigma*s) with free-dim accumulation,
   PE reduces across partitions with a ones matmul.
 - Host sums the 8 partial sums and divides by 3M.
"""
import sys
sys.path.insert(0, "/opt/trn_rl_repo")
import numpy as np

import concourse.bacc as bacc
import concourse.bass as bass
import concourse.mybir as mybir
from concourse.tile import TileContext
from concourse.vector_clock import ScopedClock
from concourse.bass_utils import run_bass_kernel_spmd

N_NODES = 500_000
D = 128
E_POS = 1_000_000
E_NEG = 2_000_000
N_CORES = 8
NSEG = 16
SEG = N_NODES // NSEG            # 31250 real rows per segment
SEG_PAD = SEG + 1                # + special row (+1s at SEG)
SP = SEG                         # local index of the +1 row
MACRO = 8192                     # edges per macro tile
# Max indices per dma_gather call: hard ucode limit of 1024 descriptors per
# call (2048 crashes even with an enlarged carveout; verified on HW). The
# enlarged carveout (64KB/partition = 4096 descs/queue) lets several 1024-desc
# calls queue per ring for deeper pipelining.
DMA_SCRATCH = 65536
GMAX = 1024
BUFS = 2                         # gather tile double-buffering depth
SRC_RUNS = False                 # run-compress src side of dst-singles region
LEVEL = 9                        # debug: 1=gathers only, 2=+mult/reduce, 9=full
IDX_UPFRONT = False              # load whole idx arrays before the macro loop

f16 = mybir.dt.float16
i16 = mybir.dt.int16
f32 = mybir.dt.float32

_MAX_DRAIN_WAITS = 1


class _SafeTileContext(TileContext):
    """Tail drain emits one wait per instruction (walrus rejects >2)."""

    def _drain_and_barrier(self, tick_clock, wait_clock):
        nc = self.nc
        probe = nc.sync.nop()
        wait_clock.add_sem_waits(
            probe.ins, ScopedClock({None: tick_clock.global_clock})
        )
        si = probe.ins.sync_info
        waits = list(si.on_wait or []) if si else []
        if len(waits) > _MAX_DRAIN_WAITS:
            by_name = {h.name: h for h in self.sems.allocated().values()}
            si.on_wait = []
            probe.ins.sync_info = si
            for sw in waits:
                w = nc.sync.nop(nofuse=True)
                w.wait_op(by_name[sw.ant_name], sw.wait_value, "sem-ge")
        nc.sync.drain()
        nc.all_engine_barrier()
        assert self.sems is not None
        popped = nc._tile_sem_poison_stack.pop()
        assert popped is self._sem_poison
        nc.clear_and_free_semaphores(list(self.sems.allocated().values()))
        nc.all_engine_barrier()


def _wrap16(flat):
    """[n] int16 -> [128, n//16] wrapped-in-16, replicated across Q7 cores."""
    n = flat.shape[0]
    blk = flat.reshape(n // 16, 16).T
    return np.ascontiguousarray(np.tile(blk, (8, 1)).astype(np.int16))


def _wrap128(flat, dtype):
    """[n] -> [128, n//128] with slot e at (e%128, e//128)."""
    n = flat.shape[0]
    return np.ascontiguousarray(flat.reshape(n // 128, 128).T.astype(dtype))


NPOS = 2 * NSEG  # schedule positions per core
CLASSES = (4, 2, 1)  # descriptor run lengths (rows per descriptor)


def _chunk_group(dl_vals):
    """Decompose a group's dst_local multiset into runs of consecutive
    distinct values, chopped into chunks of length 4 / 2 / 1.

    Returns {L: (starts[nL], edges[nL, L])} where edges holds indices into
    dl_vals (i.e. local edge ids) and starts the first dst value of each
    chunk (chunk covers starts..starts+L-1 in order).
    """
    n = dl_vals.shape[0]
    order_v = np.argsort(dl_vals, kind="stable")
    sv = dl_vals[order_v]
    change = np.r_[True, sv[1:] != sv[:-1]]
    first_idx = np.flatnonzero(change)
    counts = np.diff(np.r_[first_idx, n])
    level = np.arange(n) - np.repeat(first_idx, counts)
    order2 = np.lexsort((sv, level))
    v2 = sv[order2]
    lev2 = level[order2]
    edge2 = order_v[order2]
    brk = np.r_[True, (lev2[1:] != lev2[:-1]) | (v2[1:] != v2[:-1] + 1)]
    run_id = np.cumsum(brk) - 1
    run_len = np.bincount(run_id)
    run_first = np.r_[0, np.cumsum(run_len)[:-1]]
    o = np.arange(n) - run_first[run_id]
    rl = run_len[run_id]
    n4p = 4 * (rl // 4)
    cls = np.where(o < n4p, 4, np.where(o < n4p + 2 * ((rl % 4) // 2), 2, 1))
    out = {}
    for L in CLASSES:
        m = cls == L
        vL = v2[m]
        eL = edge2[m]
        k = vL.shape[0] // L
        vL = vL.reshape(k, L)
        out[L] = (vL[:, 0].copy(), eL.reshape(k, L))
    return out


def _assign(src, dst, sigma):
    """Diagonal bucket-to-core assignment + dst run-length compression.

    Bucket (s_seg, d_seg) goes WHOLLY to core ((d_seg - s_seg) % 16) // 2;
    within a core, schedule position p = 2*s_seg + ((d_seg - s_seg) % 2).
    All cores run the identical program: at position p = 2s+j the src base
    is seg s of the plain table, the dst base is rotated position
    (s+j) % 16 of a per-core rotated table (rotation = 2c segments).

    Within each position, edges are dst-sorted and packed into run-length
    descriptor chunks of 4/2/1 consecutive dst rows; shared per-position
    class quotas (q4, q2, q1) are chosen so every core can fill q4/q2 with
    real runs (min over cores, excess demoted downward) and q1 absorbs the
    remainder (max over cores, padded with +1-row singles).

    Returns (sched, n_tot, src_locals, sigs, dst_descs) where sched is a
    list of (q4, q2, q1) per position and dst_descs are per-core
    descriptor-stream start indices (int16).
    """
    s_seg = (src // SEG).astype(np.int32)
    d_seg = (dst // SEG).astype(np.int32)
    diag = (d_seg - s_seg) % NSEG
    core = diag // 2
    pos = 2 * s_seg + (diag % 2)
    src_l = (src % SEG).astype(np.int16)
    dst_l = (dst % SEG).astype(np.int16)

    key = (core * NPOS + pos).astype(np.int64)
    order = np.argsort(key, kind="stable")
    gcounts = np.bincount(key, minlength=N_CORES * NPOS)
    gstarts = np.zeros(N_CORES * NPOS + 1, dtype=np.int64)
    np.cumsum(gcounts, out=gstarts[1:])

    # per (core, pos) chunk decomposition
    chunks = {}
    for c in range(N_CORES):
        for p in range(NPOS):
            g = c * NPOS + p
            eid = order[gstarts[g]:gstarts[g + 1]]
            ch = _chunk_group(dst_l[eid].astype(np.int32))
            chunks[(c, p)] = {L: (st, eid[ed]) for L, (st, ed) in ch.items()}

    # shared quotas per position with demotion
    sched = []
    for p in range(NPOS):
        q4 = min(chunks[(c, p)][4][0].shape[0] for c in range(N_CORES))
        q4 = (q4 // 128) * 128
        n2p = []
        for c in range(N_CORES):
            s4, e4 = chunks[(c, p)][4]
            k = s4.shape[0] - q4  # demote last k 4-chunks to 2-chunks
            s2, e2 = chunks[(c, p)][2]
            if k > 0:
                s2x = np.concatenate([s2, s4[q4:], s4[q4:] + 2])
                e2x = np.concatenate([e2, e4[q4:, 0:2], e4[q4:, 2:4]])
            else:
                s2x, e2x = s2, e2
            chunks[(c, p)][4] = (s4[:q4], e4[:q4])
            chunks[(c, p)][2] = (s2x, e2x)
            n2p.append(s2x.shape[0])
        q2 = (min(n2p) // 256) * 256
        n1p = []
        for c in range(N_CORES):
            s2, e2 = chunks[(c, p)][2]
            k = s2.shape[0] - q2
            s1, e1 = chunks[(c, p)][1]
            e1 = e1.reshape(-1)
            if k > 0:
                s1 = np.concatenate([s1, s2[q2:], s2[q2:] + 1])
                e1 = np.concatenate([e1, e2[q2:, 0], e2[q2:, 1]])
            chunks[(c, p)][2] = (s2[:q2], e2[:q2])
            chunks[(c, p)][1] = (s1, e1)
            n1p.append(s1.shape[0])

        # src-side run compression of the dst-singles region: those edges
        # can sit in any order, so sort by src and chunk the SRC values.
        schunks = {}
        sq4 = None
        for c in range(N_CORES):
            s1, e1 = chunks[(c, p)][1]
            if SRC_RUNS:
                ch = _chunk_group(src_l[e1].astype(np.int32))
                schunks[c] = {L: (st, e1[ed]) for L, (st, ed) in ch.items()}
            else:
                empty = np.zeros(0, dtype=np.int64)
                schunks[c] = {
                    4: (empty, empty.reshape(0, 4)),
                    2: (empty, empty.reshape(0, 2)),
                    1: (src_l[e1].astype(np.int32), e1),
                }
        sq4 = min(schunks[c][4][0].shape[0] for c in range(N_CORES))
        sq4 = (sq4 // 128) * 128
        sn2 = []
        for c in range(N_CORES):
            s4, e4 = schunks[c][4]
            k = s4.shape[0] - sq4
            s2x, e2x = schunks[c][2]
            if k > 0:
                s2x = np.concatenate([s2x, s4[sq4:], s4[sq4:] + 2])
                e2x = np.concatenate([e2x, e4[sq4:, 0:2], e4[sq4:, 2:4]])
            schunks[c][4] = (s4[:sq4], e4[:sq4])
            schunks[c][2] = (s2x, e2x)
            sn2.append(s2x.shape[0])
        sq2 = (min(sn2) // 256) * 256
        sn1 = []
        for c in range(N_CORES):
            s2x, e2x = schunks[c][2]
            k = s2x.shape[0] - sq2
            s1x, e1x = schunks[c][1]
            e1x = e1x.reshape(-1)
            if k > 0:
                s1x = np.concatenate([s1x, s2x[sq2:], s2x[sq2:] + 1])
                e1x = np.concatenate([e1x, e2x[sq2:, 0], e2x[sq2:, 1]])
            schunks[c][2] = (s2x[:sq2], e2x[:sq2])
            schunks[c][1] = (s1x, e1x.reshape(-1, 1))
            sn1.append(s1x.shape[0])
        sq1 = ((max(sn1) + 511) // 512) * 512
        for c in range(N_CORES):
            chunks[(c, p)]["s"] = schunks[c]
        sched.append((int(q4), int(q2), int(sq4), int(sq2), int(sq1)))

    def pos_slots(e):
        q4, q2, sq4, sq2, sq1 = e
        return 4 * q4 + 2 * q2 + (4 * sq4 + 2 * sq2 + sq1)

    n_tot = sum(pos_slots(e) for e in sched)
    ndesc_tot = sum(q4 + q2 + (4 * sq4 + 2 * sq2 + sq1)
                    for q4, q2, sq4, sq2, sq1 in sched)
    nsdesc_tot = sum(sq4 + sq2 + sq1 for _, _, sq4, sq2, sq1 in sched)

    src_locals, sigs, dst_descs, srun_descs = [], [], [], []
    for c in range(N_CORES):
        sl = np.full(n_tot, SP, dtype=np.int16)
        sg = np.full(n_tot, -1.0, dtype=np.float16)
        dd = np.full(ndesc_tot, SP, dtype=np.int16)
        sd = np.full(nsdesc_tot, SP, dtype=np.int16)
        P0, Dc, Sc = 0, 0, 0
        for p in range(NPOS):
            q4, q2, sq4, sq2, sq1 = sched[p]
            # dst 4/2 regions: slot-stream src, desc-stream dst
            for L, q in ((4, q4), (2, q2)):
                st, ed = chunks[(c, p)][L]
                nd = st.shape[0]
                if nd > 0:
                    dd[Dc:Dc + nd] = st.astype(np.int16)
                    d = np.arange(nd, dtype=np.int64)
                    base = P0 + (d % 128) + 128 * L * (d // 128)
                    for r in range(L):
                        slot = base + 128 * r
                        sl[slot] = src_l[ed[:, r]]
                        sg[slot] = sigma[ed[:, r]]
                P0 += L * q
                Dc += q
            # dst singles region, subdivided by src run classes: src via
            # run descriptors, dst via per-slot singles descriptors.
            B1, D1 = P0, Dc
            for L, q in ((4, sq4), (2, sq2), (1, sq1)):
                st, ed = chunks[(c, p)]["s"][L]
                nd = st.shape[0]
                if nd > 0:
                    sd[Sc:Sc + nd] = st.astype(np.int16)
                    d = np.arange(nd, dtype=np.int64)
                    base = P0 + (d % 128) + 128 * L * (d // 128)
                    for r in range(L):
                        slot = base + 128 * r
                        edge = ed[:, r]
                        sl[slot] = src_l[edge]
                        sg[slot] = sigma[edge]
                        dd[D1 + (slot - B1)] = dst_l[edge]
                P0 += L * q
                Sc += q
            Dc = D1 + (P0 - B1)
        src_locals.append(sl)
        sigs.append(sg)
        dst_descs.append(dd)
        srun_descs.append(sd)
    return sched, n_tot, src_locals, sigs, dst_descs, srun_descs


def _ops(sched):
    """Build the gather op list from the class-quota schedule.

    Returns (n_tot, ndesc_tot, nsdesc_tot, macros); macros is a list of
    (m_start, m_n, src_ops, dst_ops).
    src ops: ('slot', seg, start, n)            per-slot gather, plain table
             ('run', L, seg, start, nd, soff)   L-row runs, plain table
    dst ops: (L, seg, start, nd, doff)          L-row runs, rotated table
    Run descriptors write slots start + d%128 + 128*(L*(d//128) + r).
    """
    n_tot = 0
    ndesc_tot = 0
    nsdesc_tot = 0
    dregions = []   # (L, k, slot_start, slot_end, desc_start)
    sregions = []   # ('slot', seg, a, b) | ('run', L, seg, a, b, soff)
    P0, Dc, Sc = 0, 0, 0
    for p in range(NPOS):
        q4, q2, sq4, sq2, sq1 = sched[p]
        s, j = p // 2, p % 2
        k = (s + j) % NSEG
        # dst 4/2 regions (src = slot stream)
        a42 = P0
        for L, q in ((4, q4), (2, q2)):
            if q > 0:
                dregions.append((L, k, P0, P0 + L * q, Dc))
            P0 += L * q
            Dc += q
        if not SRC_RUNS:
            # src side is the plain slot stream for the whole position
            B1 = P0 + 4 * sq4 + 2 * sq2 + sq1
            if B1 > a42:
                sregions.append(("slot", s, a42, B1))
            if B1 > P0:
                dregions.append((1, k, P0, B1, Dc))
                Dc += B1 - P0
            Sc += sq4 + sq2 + sq1
            P0 = B1
            continue
        if P0 > a42:
            sregions.append(("slot", s, a42, P0))
        # dst singles region (src = run descriptors)
        B1 = P0
        for L, q in ((4, sq4), (2, sq2), (1, sq1)):
            if q > 0:
                sregions.append(("run", L, s, P0, P0 + L * q, Sc))
            P0 += L * q
            Sc += q
        if P0 > B1:
            dregions.append((1, k, B1, P0, Dc))
            Dc += P0 - B1
    n_tot, ndesc_tot, nsdesc_tot = P0, Dc, Sc

    def cut_dst(m0, m1):
        ops = []
        for (L, k, a, b, doff) in dregions:
            lo, hi = max(a, m0), min(b, m1)
            if lo >= hi:
                continue
            pos = lo
            while pos < hi:
                nd = min(GMAX, (hi - pos) // L)
                ops.append((L, k, pos, nd, doff + (pos - a) // L))
                pos += nd * L
        return ops

    def cut_src(m0, m1):
        ops = []
        for reg in sregions:
            if reg[0] == "slot":
                _, seg, a, b = reg
                lo, hi = max(a, m0), min(b, m1)
                pos = lo
                while pos < hi:
                    n = min(GMAX, hi - pos)
                    ops.append(("slot", seg, pos, n))
                    pos += n
            else:
                _, L, seg, a, b, soff = reg
                lo, hi = max(a, m0), min(b, m1)
                pos = lo
                while pos < hi:
                    nd = min(GMAX, (hi - pos) // L)
                    ops.append(("run", L, seg, pos, nd, soff + (pos - a) // L))
                    pos += nd * L
        return ops

    macros = []
    m0 = 0
    while m0 < n_tot:
        m1 = min(m0 + MACRO, n_tot)
        macros.append((m0, m1 - m0, cut_src(m0, m1), cut_dst(m0, m1)))
        m0 = m1
    return n_tot, ndesc_tot, nsdesc_tot, macros


def _build_program(sched, repeat=1):
    n_tot, ndesc_tot, nsdesc_tot, macros = _ops(sched)
    w16 = n_tot // 16
    wd16 = ndesc_tot // 16
    ws16 = nsdesc_tot // 16
    cols = n_tot // 128

    nc = bacc.Bacc("TRN2", target_bir_lowering=False, num_swdge_queues=4,
                   dynamic_dma_scratch_size=DMA_SCRATCH)
    h16 = nc.dram_tensor("h16", [NSEG * SEG_PAD, D], f16, kind="ExternalInput")
    hrot = nc.dram_tensor("hrot", [NSEG * SEG_PAD, D], f16, kind="ExternalInput")
    d_src = nc.dram_tensor("src16", [128, w16], i16, kind="ExternalInput")
    d_srun = nc.dram_tensor("srun16", [128, ws16], i16, kind="ExternalInput")
    d_dst = nc.dram_tensor("dst16", [128, wd16], i16, kind="ExternalInput")
    d_sig = nc.dram_tensor("sig", [128, cols], f16, kind="ExternalInput")
    out = nc.dram_tensor("partial", [1, 1], f32, kind="ExternalOutput")

    AF = mybir.ActivationFunctionType
    with _SafeTileContext(nc) as tc:
        with (
            tc.tile_pool(name="idxp", bufs=3) as idxp,
            tc.tile_pool(name="srcg", bufs=BUFS) as srcg,
            tc.tile_pool(name="dstg", bufs=BUFS) as dstg,
            tc.tile_pool(name="work", bufs=2) as workp,
            tc.tile_pool(name="persist", bufs=1) as persist,
            tc.tile_pool(name="fin", bufs=1) as finp,
            tc.tile_pool(name="psum", bufs=1, space="PSUM") as psump,
        ):
            nregs = {}

            def nreg(n):
                if n not in nregs:
                    nregs[n] = nc.gpsimd.snap(n)
                return nregs[n]

            qctr = [0]

            def nextq():
                qctr[0] = (qctr[0] + 1) % 4
                return qctr[0]

            sig = persist.tile([128, cols], f16, tag="sig")
            nc.sync.dma_start(out=sig[:], in_=d_sig[:, :])
            score = persist.tile([128, cols], f16, tag="score")
            if IDX_UPFRONT:
                si_all = persist.tile([128, w16], i16, tag="si_all")
                di_all = persist.tile([128, w16], i16, tag="di_all")
                nc.sync.dma_start(out=si_all[:], in_=d_src[:, :])
                nc.sync.dma_start(out=di_all[:], in_=d_dst[:, :])

            def run_ap(table, seg, L):
                base = table[seg * SEG_PAD:, :]
                if L == 1:
                    return base
                return bass.AP(
                    base.tensor, base.offset,
                    [[D, (NSEG - seg) * SEG_PAD - (L - 1)], [1, L * D]])

            for _rep in range(repeat):
                for (m0, m_n, sops, dops) in macros:
                    mcols = m_n // 128
                    desc_lo = dops[0][4]
                    desc_hi = dops[-1][4] + dops[-1][3]
                    sruns = [op for op in sops if op[0] == "run"]
                    si = idxp.tile([128, MACRO // 16], i16, tag="si")
                    di = idxp.tile([128, MACRO // 16], i16, tag="di")
                    nc.sync.dma_start(
                        out=si[:, :m_n // 16], in_=d_src[:, m0 // 16:(m0 + m_n) // 16])
                    nc.sync.dma_start(
                        out=di[:, :(desc_hi - desc_lo) // 16],
                        in_=d_dst[:, desc_lo // 16:desc_hi // 16])
                    if sruns:
                        s_lo = sruns[0][5]
                        s_hi = sruns[-1][5] + sruns[-1][4]
                        sr = idxp.tile([128, MACRO // 16], i16, tag="sr")
                        nc.sync.dma_start(
                            out=sr[:, :(s_hi - s_lo) // 16],
                            in_=d_srun[:, s_lo // 16:s_hi // 16])

                    st = srcg.tile([128, MACRO], f16, tag="st")
                    dt_ = dstg.tile([128, MACRO], f16, tag="dt")
                    for op in sops:
                        if op[0] == "slot":
                            _, seg, start, n = op
                            o = start - m0
                            iv = si[:, o // 16:(o + n) // 16]
                            nc.gpsimd.dma_gather(
                                st[:, o:o + n].rearrange("p (c d) -> p c d", d=D),
                                h16[seg * SEG_PAD:, :],
                                iv,
                                n, nreg(n), D, elem_step=D, queue_num=nextq(),
                            )
                        else:
                            _, L, seg, start, nd, soff = op
                            o = start - m0
                            lo = soff - s_lo
                            iv = sr[:, lo // 16:(lo + nd) // 16]
                            nc.gpsimd.dma_gather(
                                st[:, o:o + nd * L].rearrange(
                                    "p (c d) -> p c d", d=L * D),
                                run_ap(h16, seg, L),
                                iv,
                                nd, nreg(nd), L * D, elem_step=D,
                                queue_num=nextq(),
                            )
                    for (L, seg, start, nd, doff) in dops:
                        o = start - m0
                        lo = doff - desc_lo
                        iv = di[:, lo // 16:(lo + nd) // 16]
                        nc.gpsimd.dma_gather(
                            dt_[:, o:o + nd * L].rearrange(
                                "p (c d) -> p c d", d=L * D),
                            run_ap(hrot, seg, L),
                            iv,
                            nd, nreg(nd), L * D, elem_step=D,
                            queue_num=nextq(),
                        )
                    if LEVEL < 2:
                        continue
                    prod = workp.tile([128, MACRO], f16, tag="prod")
                    nc.vector.tensor_tensor(
                        out=prod[:, :m_n], in0=st[:, :m_n], in1=dt_[:, :m_n],
                        op=mybir.AluOpType.mult,
                    )
                    with nc.allow_low_precision(
                            reason="fp16 score ok for 2e-2 tolerance"):
                        nc.vector.tensor_reduce(
                            out=score[:, m0 // 128:m0 // 128 + mcols],
                            in_=prod[:, :m_n].rearrange("p (c d) -> p c d", d=D),
                            axis=mybir.AxisListType.X,
                            op=mybir.AluOpType.add,
                        )

                # loss_e = ln(1+exp(-|s|)) + relu(sigma*s)
                if LEVEL < 2:
                    res = finp.tile([1, 1], f32, tag="res")
                    nc.vector.memset(res[:], 1.0)
                    continue
                acc = finp.tile([128, 2], f32, tag="acc")
                t1 = finp.tile([128, cols], f16, tag="sp_t1")
                t2 = finp.tile([128, cols], f16, tag="sp_t2")
                nc.scalar.activation(out=t1[:], in_=score[:], func=AF.Abs)
                nc.scalar.activation(out=t2[:], in_=t1[:], func=AF.Exp, scale=-1.0)
                nc.scalar.activation(out=t1[:], in_=t2[:], func=AF.Ln, bias=1.0,
                                     accum_out=acc[:, 0:1])
                nc.vector.tensor_tensor(out=t2[:], in0=score[:], in1=sig[:],
                                        op=mybir.AluOpType.mult)
                nc.scalar.activation(out=t2[:], in_=t2[:], func=AF.Relu,
                                     accum_out=acc[:, 1:2])

                acc1 = finp.tile([128, 1], f32, tag="acc1")
                nc.vector.tensor_reduce(out=acc1[:], in_=acc[:],
                                        axis=mybir.AxisListType.X,
                                        op=mybir.AluOpType.add)
                ones = finp.tile([128, 1], f32, tag="ones")
                nc.vector.memset(ones[:], 1.0)
                ps = psump.tile([1, 1], f32, tag="ps")
                nc.tensor.matmul(ps[:], lhsT=acc1[:], rhs=ones[:],
                                 start=True, stop=True)
                res = finp.tile([1, 1], f32, tag="res")
                nc.vector.tensor_copy(out=res[:], in_=ps[:])
            nc.sync.dma_start(out=out[:, :], in_=res[:])
    nc.finalize()
    return nc


def _pack_table(h):
    """fp32 [N, D] -> fp16 [NSEG*SEG_PAD, D] with +1 special rows."""
    t = np.empty((NSEG * SEG_PAD, D), dtype=np.float16)
    hv = h.astype(np.float16).reshape(NSEG, SEG, D)
    for s in range(NSEG):
        t[s * SEG_PAD:s * SEG_PAD + SEG] = hv[s]
        t[s * SEG_PAD + SP] = np.float16(1.0)
    return t


def _prepare(h, pos_src, pos_dst, neg_src, neg_dst):
    """Host-side planning: returns (quota, in_maps)."""
    h = np.asarray(h)
    src = np.concatenate([np.asarray(pos_src), np.asarray(neg_src)]).astype(np.int64)
    dst = np.concatenate([np.asarray(pos_dst), np.asarray(neg_dst)]).astype(np.int64)
    sigma = np.concatenate([
        np.full(E_POS, -1.0, dtype=np.float16),
        np.full(E_NEG, +1.0, dtype=np.float16),
    ])

    table = _pack_table(h)
    table_segs = table.reshape(NSEG, SEG_PAD, D)
    sched, n_tot, src_l, sig_l, dst_d, srun_d = _assign(src, dst, sigma)

    in_maps = []
    for c in range(N_CORES):
        rot = np.ascontiguousarray(
            table_segs[(np.arange(NSEG) + 2 * c) % NSEG].reshape(-1, D))
        in_maps.append({
            "h16": table,
            "hrot": rot,
            "src16": _wrap16(src_l[c]),
            "srun16": _wrap16(srun_d[c]),
            "dst16": _wrap16(dst_d[c]),
            "sig": _wrap128(sig_l[c], np.float16),
        })
    return sched, in_maps


def kernel(h, pos_src, pos_dst, neg_src, neg_dst):
    sched, in_maps = _prepare(h, pos_src, pos_dst, neg_src, neg_dst)
    nc = _build_program(sched)
    res = run_bass_kernel_spmd(nc, in_maps, core_ids=list(range(N_CORES)))
    total = float(sum(float(r["partial"][0, 0]) for r in res.results))
    loss = total / float(E_POS + E_NEG)
    return np.float32(loss)


if __name__ == "__main__":
    rng = np.random.default_rng(0)
    h = rng.standard_normal((N_NODES, D)).astype(np.float32)
    a = rng.integers(0, N_NODES, size=E_POS)
    b = rng.integers(0, N_NODES, size=E_POS)
    c_ = rng.integers(0, N_NODES, size=E_NEG)
    d_ = rng.integers(0, N_NODES, size=E_NEG)
    got = kernel(h, a, b, c_, d_)
    s1 = np.einsum("ij,ij->i", h[a].astype(np.float32), h[b].astype(np.float32))
    s2 = np.einsum("ij,ij->i", h[c_].astype(np.float32), h[d_].astype(np.float32))
    exp = (np.logaddexp(0, -s1).sum() + np.logaddexp(0, s2).sum()) / 3e6
    print("got", got, "exp", exp, "rel", abs(got - exp) / abs(exp))



# revision 29
# speedup vs baseline: 1.1234x; 1.1234x over previous
"""Trainium2 Bass kernel for GNN link-prediction BCE loss.

loss = mean over 3M edges of BCE-with-logits(dot(h[src], h[dst]), label)
     = [ sum_pos softplus(-s) + sum_neg softplus(+s) ] / 3M

The workload is descriptor-rate bound on the SWDGE gather path (measured
~1.8ns fixed + ~0.9ns/256B per descriptor, 1024 descs/call max), so the
kernel minimizes DESCRIPTOR COUNT, not call count or bytes.

Strategy (8 NeuronCores, SPMD):
 - h is cast to fp16 and repacked into 16 segments of 31250 rows (+1s
   special pad row per segment); pad edges hit (+1,+1) rows -> score 128
   -> exactly 0 loss with sigma=-1.
 - Diagonal bucket assignment: bucket (s_seg, d_seg) goes wholly to core
   ((d_seg-s_seg)%16)//2, giving every core 32 whole buckets. All cores
   run the IDENTICAL program: src gathers address the plain table, dst
   gathers a per-core table whose segments are rotated by 2c, making the
   schedule core-independent while the data differs.
 - Run-length-compressed gathers: within each bucket, dst values are
   sorted and packed into descriptors covering 4/2/1 CONSECUTIVE rows
   (elem_size=L*256B with elem_step=256B overlapping-window APs), so one
   descriptor serves up to 4 edges. Shared per-position class quotas
   (min across cores, excess demoted 4->2->1) keep the program SPMD.
 - The leftover dst-singles region is re-sorted by src value and the SRC
   side is run-length compressed there the same way (the dst side of
   those slots stays per-edge singles).
 - Macro-tiles of 4096 edge slots: gathers fill src/dst row tiles, DVE
   multiplies + reduces to fp16 scores, scalar engine computes stable
   softplus ln(1+exp(-|s|)) + relu(sigma*s) with free-dim accumulation,
   PE reduces across partitions with a ones matmul.
 - Host sums the 8 partial sums and divides by 3M.
"""
import sys
sys.path.insert(0, "/opt/trn_rl_repo")
import numpy as np

import concourse.bacc as bacc
import concourse.bass as bass
import concourse.mybir as mybir
from concourse.tile import TileContext
from concourse.vector_clock import ScopedClock
from concourse.bass_utils import run_bass_kernel_spmd

N_NODES = 500_000
D = 128
E_POS = 1_000_000
E_NEG = 2_000_000
N_CORES = 8
NSEG = 16
SEG = N_NODES // NSEG            # 31250 real rows per segment
SEG_PAD = SEG + 1                # + special row (+1s at SEG)
SP = SEG                         # local index of the +1 row
MACRO = 4096                     # edges per macro tile
# Max indices per dma_gather call: hard ucode limit of 1024 descriptors per
# call (2048 crashes even with an enlarged carveout; verified on HW). The
# enlarged carveout (64KB/partition = 4096 descs/queue) lets several 1024-desc
# calls queue per ring for deeper pipelining.
DMA_SCRATCH = 65536
GMAX = 1024
BUFS = 3                         # gather tile double-buffering depth
SRC_RUNS = False                 # run-compress src side of dst-singles region
LEVEL = 9                        # debug: 1=gathers only, 2=+mult/reduce, 9=full
IDX_UPFRONT = False              # load whole idx arrays before the macro loop

f16 = mybir.dt.float16
i16 = mybir.dt.int16
f32 = mybir.dt.float32

_MAX_DRAIN_WAITS = 1


class _SafeTileContext(TileContext):
    """Tail drain emits one wait per instruction (walrus rejects >2)."""

    def _drain_and_barrier(self, tick_clock, wait_clock):
        nc = self.nc
        probe = nc.sync.nop()
        wait_clock.add_sem_waits(
            probe.ins, ScopedClock({None: tick_clock.global_clock})
        )
        si = probe.ins.sync_info
        waits = list(si.on_wait or []) if si else []
        if len(waits) > _MAX_DRAIN_WAITS:
            by_name = {h.name: h for h in self.sems.allocated().values()}
            si.on_wait = []
            probe.ins.sync_info = si
            for sw in waits:
                w = nc.sync.nop(nofuse=True)
                w.wait_op(by_name[sw.ant_name], sw.wait_value, "sem-ge")
        nc.sync.drain()
        nc.all_engine_barrier()
        assert self.sems is not None
        popped = nc._tile_sem_poison_stack.pop()
        assert popped is self._sem_poison
        nc.clear_and_free_semaphores(list(self.sems.allocated().values()))
        nc.all_engine_barrier()


def _wrap16(flat):
    """[n] int16 -> [128, n//16] wrapped-in-16, replicated across Q7 cores."""
    n = flat.shape[0]
    blk = flat.reshape(n // 16, 16).T
    return np.ascontiguousarray(np.tile(blk, (8, 1)).astype(np.int16))


def _wrap128(flat, dtype):
    """[n] -> [128, n//128] with slot e at (e%128, e//128)."""
    n = flat.shape[0]
    return np.ascontiguousarray(flat.reshape(n // 128, 128).T.astype(dtype))


NPOS = 2 * NSEG  # schedule positions per core
CLASSES = (4, 2, 1)  # descriptor run lengths (rows per descriptor)


def _chunk_group(dl_vals):
    """Decompose a group's dst_local multiset into runs of consecutive
    distinct values, chopped into chunks of length 4 / 2 / 1.

    Returns {L: (starts[nL], edges[nL, L])} where edges holds indices into
    dl_vals (i.e. local edge ids) and starts the first dst value of each
    chunk (chunk covers starts..starts+L-1 in order).
    """
    n = dl_vals.shape[0]
    order_v = np.argsort(dl_vals, kind="stable")
    sv = dl_vals[order_v]
    change = np.r_[True, sv[1:] != sv[:-1]]
    first_idx = np.flatnonzero(change)
    counts = np.diff(np.r_[first_idx, n])
    level = np.arange(n) - np.repeat(first_idx, counts)
    order2 = np.lexsort((sv, level))
    v2 = sv[order2]
    lev2 = level[order2]
    edge2 = order_v[order2]
    brk = np.r_[True, (lev2[1:] != lev2[:-1]) | (v2[1:] != v2[:-1] + 1)]
    run_id = np.cumsum(brk) - 1
    run_len = np.bincount(run_id)
    run_first = np.r_[0, np.cumsum(run_len)[:-1]]
    o = np.arange(n) - run_first[run_id]
    rl = run_len[run_id]
    n4p = 4 * (rl // 4)
    cls = np.where(o < n4p, 4, np.where(o < n4p + 2 * ((rl % 4) // 2), 2, 1))
    out = {}
    for L in CLASSES:
        m = cls == L
        vL = v2[m]
        eL = edge2[m]
        k = vL.shape[0] // L
        vL = vL.reshape(k, L)
        out[L] = (vL[:, 0].copy(), eL.reshape(k, L))
    return out


def _assign(src, dst, sigma):
    """Diagonal bucket-to-core assignment + dst run-length compression.

    Bucket (s_seg, d_seg) goes WHOLLY to core ((d_seg - s_seg) % 16) // 2;
    within a core, schedule position p = 2*s_seg + ((d_seg - s_seg) % 2).
    All cores run the identical program: at position p = 2s+j the src base
    is seg s of the plain table, the dst base is rotated position
    (s+j) % 16 of a per-core rotated table (rotation = 2c segments).

    Within each position, edges are dst-sorted and packed into run-length
    descriptor chunks of 4/2/1 consecutive dst rows; shared per-position
    class quotas (q4, q2, q1) are chosen so every core can fill q4/q2 with
    real runs (min over cores, excess demoted downward) and q1 absorbs the
    remainder (max over cores, padded with +1-row singles).

    Returns (sched, n_tot, src_locals, sigs, dst_descs) where sched is a
    list of (q4, q2, q1) per position and dst_descs are per-core
    descriptor-stream start indices (int16).
    """
    s_seg = (src // SEG).astype(np.int32)
    d_seg = (dst // SEG).astype(np.int32)
    diag = (d_seg - s_seg) % NSEG
    core = diag // 2
    pos = 2 * s_seg + (diag % 2)
    src_l = (src % SEG).astype(np.int16)
    dst_l = (dst % SEG).astype(np.int16)

    key = (core * NPOS + pos).astype(np.int64)
    order = np.argsort(key, kind="stable")
    gcounts = np.bincount(key, minlength=N_CORES * NPOS)
    gstarts = np.zeros(N_CORES * NPOS + 1, dtype=np.int64)
    np.cumsum(gcounts, out=gstarts[1:])

    # per (core, pos) chunk decomposition
    chunks = {}
    for c in range(N_CORES):
        for p in range(NPOS):
            g = c * NPOS + p
            eid = order[gstarts[g]:gstarts[g + 1]]
            ch = _chunk_group(dst_l[eid].astype(np.int32))
            chunks[(c, p)] = {L: (st, eid[ed]) for L, (st, ed) in ch.items()}

    # shared quotas per position with demotion
    sched = []
    for p in range(NPOS):
        q4 = min(chunks[(c, p)][4][0].shape[0] for c in range(N_CORES))
        q4 = (q4 // 128) * 128
        n2p = []
        for c in range(N_CORES):
            s4, e4 = chunks[(c, p)][4]
            k = s4.shape[0] - q4  # demote last k 4-chunks to 2-chunks
            s2, e2 = chunks[(c, p)][2]
            if k > 0:
                s2x = np.concatenate([s2, s4[q4:], s4[q4:] + 2])
                e2x = np.concatenate([e2, e4[q4:, 0:2], e4[q4:, 2:4]])
            else:
                s2x, e2x = s2, e2
            chunks[(c, p)][4] = (s4[:q4], e4[:q4])
            chunks[(c, p)][2] = (s2x, e2x)
            n2p.append(s2x.shape[0])
        q2 = (min(n2p) // 256) * 256
        n1p = []
        for c in range(N_CORES):
            s2, e2 = chunks[(c, p)][2]
            k = s2.shape[0] - q2
            s1, e1 = chunks[(c, p)][1]
            e1 = e1.reshape(-1)
            if k > 0:
                s1 = np.concatenate([s1, s2[q2:], s2[q2:] + 1])
                e1 = np.concatenate([e1, e2[q2:, 0], e2[q2:, 1]])
            chunks[(c, p)][2] = (s2[:q2], e2[:q2])
            chunks[(c, p)][1] = (s1, e1)
            n1p.append(s1.shape[0])

        # src-side run compression of the dst-singles region: those edges
        # can sit in any order, so sort by src and chunk the SRC values.
        schunks = {}
        sq4 = None
        for c in range(N_CORES):
            s1, e1 = chunks[(c, p)][1]
            if SRC_RUNS:
                ch = _chunk_group(src_l[e1].astype(np.int32))
                schunks[c] = {L: (st, e1[ed]) for L, (st, ed) in ch.items()}
            else:
                empty = np.zeros(0, dtype=np.int64)
                schunks[c] = {
                    4: (empty, empty.reshape(0, 4)),
                    2: (empty, empty.reshape(0, 2)),
                    1: (src_l[e1].astype(np.int32), e1),
                }
        sq4 = min(schunks[c][4][0].shape[0] for c in range(N_CORES))
        sq4 = (sq4 // 128) * 128
        sn2 = []
        for c in range(N_CORES):
            s4, e4 = schunks[c][4]
            k = s4.shape[0] - sq4
            s2x, e2x = schunks[c][2]
            if k > 0:
                s2x = np.concatenate([s2x, s4[sq4:], s4[sq4:] + 2])
                e2x = np.concatenate([e2x, e4[sq4:, 0:2], e4[sq4:, 2:4]])
            schunks[c][4] = (s4[:sq4], e4[:sq4])
            schunks[c][2] = (s2x, e2x)
            sn2.append(s2x.shape[0])
        sq2 = (min(sn2) // 256) * 256
        sn1 = []
        for c in range(N_CORES):
            s2x, e2x = schunks[c][2]
            k = s2x.shape[0] - sq2
            s1x, e1x = schunks[c][1]
            e1x = e1x.reshape(-1)
            if k > 0:
                s1x = np.concatenate([s1x, s2x[sq2:], s2x[sq2:] + 1])
                e1x = np.concatenate([e1x, e2x[sq2:, 0], e2x[sq2:, 1]])
            schunks[c][2] = (s2x[:sq2], e2x[:sq2])
            schunks[c][1] = (s1x, e1x.reshape(-1, 1))
            sn1.append(s1x.shape[0])
        sq1 = ((max(sn1) + 511) // 512) * 512
        for c in range(N_CORES):
            chunks[(c, p)]["s"] = schunks[c]
        sched.append((int(q4), int(q2), int(sq4), int(sq2), int(sq1)))

    def pos_slots(e):
        q4, q2, sq4, sq2, sq1 = e
        return 4 * q4 + 2 * q2 + (4 * sq4 + 2 * sq2 + sq1)

    n_tot = sum(pos_slots(e) for e in sched)
    ndesc_tot = sum(q4 + q2 + (4 * sq4 + 2 * sq2 + sq1)
                    for q4, q2, sq4, sq2, sq1 in sched)
    nsdesc_tot = sum(sq4 + sq2 + sq1 for _, _, sq4, sq2, sq1 in sched)

    src_locals, sigs, dst_descs, srun_descs = [], [], [], []
    for c in range(N_CORES):
        sl = np.full(n_tot, SP, dtype=np.int16)
        sg = np.full(n_tot, -1.0, dtype=np.float16)
        dd = np.full(ndesc_tot, SP, dtype=np.int16)
        sd = np.full(nsdesc_tot, SP, dtype=np.int16)
        P0, Dc, Sc = 0, 0, 0
        for p in range(NPOS):
            q4, q2, sq4, sq2, sq1 = sched[p]
            # dst 4/2 regions: slot-stream src, desc-stream dst
            for L, q in ((4, q4), (2, q2)):
                st, ed = chunks[(c, p)][L]
                nd = st.shape[0]
                if nd > 0:
                    dd[Dc:Dc + nd] = st.astype(np.int16)
                    d = np.arange(nd, dtype=np.int64)
                    base = P0 + (d % 128) + 128 * L * (d // 128)
                    for r in range(L):
                        slot = base + 128 * r
                        sl[slot] = src_l[ed[:, r]]
                        sg[slot] = sigma[ed[:, r]]
                P0 += L * q
                Dc += q
            # dst singles region, subdivided by src run classes: src via
            # run descriptors, dst via per-slot singles descriptors.
            B1, D1 = P0, Dc
            for L, q in ((4, sq4), (2, sq2), (1, sq1)):
                st, ed = chunks[(c, p)]["s"][L]
                nd = st.shape[0]
                if nd > 0:
                    sd[Sc:Sc + nd] = st.astype(np.int16)
                    d = np.arange(nd, dtype=np.int64)
                    base = P0 + (d % 128) + 128 * L * (d // 128)
                    for r in range(L):
                        slot = base + 128 * r
                        edge = ed[:, r]
                        sl[slot] = src_l[edge]
                        sg[slot] = sigma[edge]
                        dd[D1 + (slot - B1)] = dst_l[edge]
                P0 += L * q
                Sc += q
            Dc = D1 + (P0 - B1)
        src_locals.append(sl)
        sigs.append(sg)
        dst_descs.append(dd)
        srun_descs.append(sd)
    return sched, n_tot, src_locals, sigs, dst_descs, srun_descs


def _ops(sched):
    """Build the gather op list from the class-quota schedule.

    Returns (n_tot, ndesc_tot, nsdesc_tot, macros); macros is a list of
    (m_start, m_n, src_ops, dst_ops).
    src ops: ('slot', seg, start, n)            per-slot gather, plain table
             ('run', L, seg, start, nd, soff)   L-row runs, plain table
    dst ops: (L, seg, start, nd, doff)          L-row runs, rotated table
    Run descriptors write slots start + d%128 + 128*(L*(d//128) + r).
    """
    n_tot = 0
    ndesc_tot = 0
    nsdesc_tot = 0
    dregions = []   # (L, k, slot_start, slot_end, desc_start)
    sregions = []   # ('slot', seg, a, b) | ('run', L, seg, a, b, soff)
    P0, Dc, Sc = 0, 0, 0
    for p in range(NPOS):
        q4, q2, sq4, sq2, sq1 = sched[p]
        s, j = p // 2, p % 2
        k = (s + j) % NSEG
        # dst 4/2 regions (src = slot stream)
        a42 = P0
        for L, q in ((4, q4), (2, q2)):
            if q > 0:
                dregions.append((L, k, P0, P0 + L * q, Dc))
            P0 += L * q
            Dc += q
        if not SRC_RUNS:
            # src side is the plain slot stream for the whole position
            B1 = P0 + 4 * sq4 + 2 * sq2 + sq1
            if B1 > a42:
                sregions.append(("slot", s, a42, B1))
            if B1 > P0:
                dregions.append((1, k, P0, B1, Dc))
                Dc += B1 - P0
            Sc += sq4 + sq2 + sq1
            P0 = B1
            continue
        if P0 > a42:
            sregions.append(("slot", s, a42, P0))
        # dst singles region (src = run descriptors)
        B1 = P0
        for L, q in ((4, sq4), (2, sq2), (1, sq1)):
            if q > 0:
                sregions.append(("run", L, s, P0, P0 + L * q, Sc))
            P0 += L * q
            Sc += q
        if P0 > B1:
            dregions.append((1, k, B1, P0, Dc))
            Dc += P0 - B1
    n_tot, ndesc_tot, nsdesc_tot = P0, Dc, Sc

    def cut_dst(m0, m1):
        ops = []
        for (L, k, a, b, doff) in dregions:
            lo, hi = max(a, m0), min(b, m1)
            if lo >= hi:
                continue
            pos = lo
            while pos < hi:
                nd = min(GMAX, (hi - pos) // L)
                ops.append((L, k, pos, nd, doff + (pos - a) // L))
                pos += nd * L
        return ops

    def cut_src(m0, m1):
        ops = []
        for reg in sregions:
            if reg[0] == "slot":
                _, seg, a, b = reg
                lo, hi = max(a, m0), min(b, m1)
                pos = lo
                while pos < hi:
                    n = min(GMAX, hi - pos)
                    ops.append(("slot", seg, pos, n))
                    pos += n
            else:
                _, L, seg, a, b, soff = reg
                lo, hi = max(a, m0), min(b, m1)
                pos = lo
                while pos < hi:
                    nd = min(GMAX, (hi - pos) // L)
                    ops.append(("run", L, seg, pos, nd, soff + (pos - a) // L))
                    pos += nd * L
        return ops

    macros = []
    m0 = 0
    while m0 < n_tot:
        m1 = min(m0 + MACRO, n_tot)
        macros.append((m0, m1 - m0, cut_src(m0, m1), cut_dst(m0, m1)))
        m0 = m1
    return n_tot, ndesc_tot, nsdesc_tot, macros


def _build_program(sched, repeat=1):
    n_tot, ndesc_tot, nsdesc_tot, macros = _ops(sched)
    w16 = n_tot // 16
    wd16 = ndesc_tot // 16
    ws16 = nsdesc_tot // 16
    cols = n_tot // 128

    nc = bacc.Bacc("TRN2", target_bir_lowering=False, num_swdge_queues=4,
                   dynamic_dma_scratch_size=DMA_SCRATCH)
    h16 = nc.dram_tensor("h16", [NSEG * SEG_PAD, D], f16, kind="ExternalInput")
    hrot = nc.dram_tensor("hrot", [NSEG * SEG_PAD, D], f16, kind="ExternalInput")
    d_src = nc.dram_tensor("src16", [128, w16], i16, kind="ExternalInput")
    d_srun = nc.dram_tensor("srun16", [128, ws16], i16, kind="ExternalInput")
    d_dst = nc.dram_tensor("dst16", [128, wd16], i16, kind="ExternalInput")
    d_sig = nc.dram_tensor("sig", [128, cols], f16, kind="ExternalInput")
    out = nc.dram_tensor("partial", [1, 1], f32, kind="ExternalOutput")

    AF = mybir.ActivationFunctionType
    with _SafeTileContext(nc) as tc:
        with (
            tc.tile_pool(name="idxp", bufs=3) as idxp,
            tc.tile_pool(name="srcg", bufs=BUFS) as srcg,
            tc.tile_pool(name="dstg", bufs=BUFS) as dstg,
            tc.tile_pool(name="work", bufs=2) as workp,
            tc.tile_pool(name="persist", bufs=1) as persist,
            tc.tile_pool(name="fin", bufs=1) as finp,
            tc.tile_pool(name="psum", bufs=1, space="PSUM") as psump,
        ):
            nregs = {}

            def nreg(n):
                if n not in nregs:
                    nregs[n] = nc.gpsimd.snap(n)
                return nregs[n]

            qctr = [0]

            def nextq():
                qctr[0] = (qctr[0] + 1) % 4
                return qctr[0]

            sig = persist.tile([128, cols], f16, tag="sig")
            nc.sync.dma_start(out=sig[:], in_=d_sig[:, :])
            score = persist.tile([128, cols], f16, tag="score")
            if IDX_UPFRONT:
                si_all = persist.tile([128, w16], i16, tag="si_all")
                di_all = persist.tile([128, w16], i16, tag="di_all")
                nc.sync.dma_start(out=si_all[:], in_=d_src[:, :])
                nc.sync.dma_start(out=di_all[:], in_=d_dst[:, :])

            def run_ap(table, seg, L):
                base = table[seg * SEG_PAD:, :]
                if L == 1:
                    return base
                return bass.AP(
                    base.tensor, base.offset,
                    [[D, (NSEG - seg) * SEG_PAD - (L - 1)], [1, L * D]])

            for _rep in range(repeat):
                for (m0, m_n, sops, dops) in macros:
                    mcols = m_n // 128
                    desc_lo = dops[0][4]
                    desc_hi = dops[-1][4] + dops[-1][3]
                    sruns = [op for op in sops if op[0] == "run"]
                    si = idxp.tile([128, MACRO // 16], i16, tag="si")
                    di = idxp.tile([128, MACRO // 16], i16, tag="di")
                    nc.sync.dma_start(
                        out=si[:, :m_n // 16], in_=d_src[:, m0 // 16:(m0 + m_n) // 16])
                    nc.sync.dma_start(
                        out=di[:, :(desc_hi - desc_lo) // 16],
                        in_=d_dst[:, desc_lo // 16:desc_hi // 16])
                    if sruns:
                        s_lo = sruns[0][5]
                        s_hi = sruns[-1][5] + sruns[-1][4]
                        sr = idxp.tile([128, MACRO // 16], i16, tag="sr")
                        nc.sync.dma_start(
                            out=sr[:, :(s_hi - s_lo) // 16],
                            in_=d_srun[:, s_lo // 16:s_hi // 16])

                    st = srcg.tile([128, MACRO], f16, tag="st")
                    dt_ = dstg.tile([128, MACRO], f16, tag="dt")
                    for op in sops:
                        if op[0] == "slot":
                            _, seg, start, n = op
                            o = start - m0
                            iv = si[:, o // 16:(o + n) // 16]
                            nc.gpsimd.dma_gather(
                                st[:, o:o + n].rearrange("p (c d) -> p c d", d=D),
                                h16[seg * SEG_PAD:, :],
                                iv,
                                n, nreg(n), D, elem_step=D, queue_num=nextq(),
                            )
                        else:
                            _, L, seg, start, nd, soff = op
                            o = start - m0
                            lo = soff - s_lo
                            iv = sr[:, lo // 16:(lo + nd) // 16]
                            nc.gpsimd.dma_gather(
                                st[:, o:o + nd * L].rearrange(
                                    "p (c d) -> p c d", d=L * D),
                                run_ap(h16, seg, L),
                                iv,
                                nd, nreg(nd), L * D, elem_step=D,
                                queue_num=nextq(),
                            )
                    for (L, seg, start, nd, doff) in dops:
                        o = start - m0
                        lo = doff - desc_lo
                        iv = di[:, lo // 16:(lo + nd) // 16]
                        nc.gpsimd.dma_gather(
                            dt_[:, o:o + nd * L].rearrange(
                                "p (c d) -> p c d", d=L * D),
                            run_ap(hrot, seg, L),
                            iv,
                            nd, nreg(nd), L * D, elem_step=D,
                            queue_num=nextq(),
                        )
                    if LEVEL < 2:
                        continue
                    prod = workp.tile([128, MACRO], f16, tag="prod")
                    nc.vector.tensor_tensor(
                        out=prod[:, :m_n], in0=st[:, :m_n], in1=dt_[:, :m_n],
                        op=mybir.AluOpType.mult,
                    )
                    with nc.allow_low_precision(
                            reason="fp16 score ok for 2e-2 tolerance"):
                        nc.vector.tensor_reduce(
                            out=score[:, m0 // 128:m0 // 128 + mcols],
                            in_=prod[:, :m_n].rearrange("p (c d) -> p c d", d=D),
                            axis=mybir.AxisListType.X,
                            op=mybir.AluOpType.add,
                        )

                # loss_e = ln(1+exp(-|s|)) + relu(sigma*s)
                if LEVEL < 2:
                    res = finp.tile([1, 1], f32, tag="res")
                    nc.vector.memset(res[:], 1.0)
                    continue
                acc = finp.tile([128, 2], f32, tag="acc")
                t1 = finp.tile([128, cols], f16, tag="sp_t1")
                t2 = finp.tile([128, cols], f16, tag="sp_t2")
                nc.scalar.activation(out=t1[:], in_=score[:], func=AF.Abs)
                nc.scalar.activation(out=t2[:], in_=t1[:], func=AF.Exp, scale=-1.0)
                nc.scalar.activation(out=t1[:], in_=t2[:], func=AF.Ln, bias=1.0,
                                     accum_out=acc[:, 0:1])
                nc.vector.tensor_tensor(out=t2[:], in0=score[:], in1=sig[:],
                                        op=mybir.AluOpType.mult)
                nc.scalar.activation(out=t2[:], in_=t2[:], func=AF.Relu,
                                     accum_out=acc[:, 1:2])

                acc1 = finp.tile([128, 1], f32, tag="acc1")
                nc.vector.tensor_reduce(out=acc1[:], in_=acc[:],
                                        axis=mybir.AxisListType.X,
                                        op=mybir.AluOpType.add)
                ones = finp.tile([128, 1], f32, tag="ones")
                nc.vector.memset(ones[:], 1.0)
                ps = psump.tile([1, 1], f32, tag="ps")
                nc.tensor.matmul(ps[:], lhsT=acc1[:], rhs=ones[:],
                                 start=True, stop=True)
                res = finp.tile([1, 1], f32, tag="res")
                nc.vector.tensor_copy(out=res[:], in_=ps[:])
            nc.sync.dma_start(out=out[:, :], in_=res[:])
    nc.finalize()
    return nc


def _pack_table(h):
    """fp32 [N, D] -> fp16 [NSEG*SEG_PAD, D] with +1 special rows."""
    t = np.empty((NSEG * SEG_PAD, D), dtype=np.float16)
    hv = h.astype(np.float16).reshape(NSEG, SEG, D)
    for s in range(NSEG):
        t[s * SEG_PAD:s * SEG_PAD + SEG] = hv[s]
        t[s * SEG_PAD + SP] = np.float16(1.0)
    return t


def _prepare(h, pos_src, pos_dst, neg_src, neg_dst):
    """Host-side planning: returns (quota, in_maps)."""
    h = np.asarray(h)
    src = np.concatenate([np.asarray(pos_src), np.asarray(neg_src)]).astype(np.int64)
    dst = np.concatenate([np.asarray(pos_dst), np.asarray(neg_dst)]).astype(np.int64)
    sigma = np.concatenate([
        np.full(E_POS, -1.0, dtype=np.float16),
        np.full(E_NEG, +1.0, dtype=np.float16),
    ])

    table = _pack_table(h)
    table_segs = table.reshape(NSEG, SEG_PAD, D)
    sched, n_tot, src_l, sig_l, dst_d, srun_d = _assign(src, dst, sigma)

    in_maps = []
    for c in range(N_CORES):
        rot = np.ascontiguousarray(
            table_segs[(np.arange(NSEG) + 2 * c) % NSEG].reshape(-1, D))
        in_maps.append({
            "h16": table,
            "hrot": rot,
            "src16": _wrap16(src_l[c]),
            "srun16": _wrap16(srun_d[c]),
            "dst16": _wrap16(dst_d[c]),
            "sig": _wrap128(sig_l[c], np.float16),
        })
    return sched, in_maps


def kernel(h, pos_src, pos_dst, neg_src, neg_dst):
    sched, in_maps = _prepare(h, pos_src, pos_dst, neg_src, neg_dst)
    nc = _build_program(sched)
    res = run_bass_kernel_spmd(nc, in_maps, core_ids=list(range(N_CORES)))
    total = float(sum(float(r["partial"][0, 0]) for r in res.results))
    loss = total / float(E_POS + E_NEG)
    return np.float32(loss)


if __name__ == "__main__":
    rng = np.random.default_rng(0)
    h = rng.standard_normal((N_NODES, D)).astype(np.float32)
    a = rng.integers(0, N_NODES, size=E_POS)
    b = rng.integers(0, N_NODES, size=E_POS)
    c_ = rng.integers(0, N_NODES, size=E_NEG)
    d_ = rng.integers(0, N_NODES, size=E_NEG)
    got = kernel(h, a, b, c_, d_)
    s1 = np.einsum("ij,ij->i", h[a].astype(np.float32), h[b].astype(np.float32))
    s2 = np.einsum("ij,ij->i", h[c_].astype(np.float32), h[d_].astype(np.float32))
    exp = (np.logaddexp(0, -s1).sum() + np.logaddexp(0, s2).sum()) / 3e6
    print("got", got, "exp", exp, "rel", abs(got - exp) / abs(exp))

